# revision 1
# baseline (speedup 1.0000x reference)
"""Multi-head attention (N=4, C=256, H=W=64, heads=8, d=32) on 8 TRN2 cores.

Sharding: core c handles batch n = c//2 and heads h0..h0+3 with
h0 = 4*(c%2).  Each core computes its [128, 4096] slice of the final
(N, C, H*W) output (channels h0*32 .. h0*32+128 of batch n).

Dataflow per core (all on device):
  xtT[C, L]   = x[n] + peT                          (DVE add, fp32)
  qT,kT[128,L] = W_{q,k}^T @ xtT  (W-chunk stationary, fp32 matmul,
                 rows = 4 heads x 32 dims, cast to bf16)
  v[L, 128]    = xtT^T @ W_v      (xtT-chunk stationary, fp32, cast bf16,
                 stored as [128 keys-in-chunk, 32 chunks x 128])
  per q-block (512 queries) x key-chunk (128 keys):
    scoresT[keys, q] = kT_chunk^T @ qT_block   4 heads row-packed (K=32)
    expT = exp(scale * scoresT)                ACT, PSUM->SBUF bf16
    outT[4hx32d, q]  += v_chunk^T @ expT       4 heads col-packed, PSUM accum
    sums[4h, q]      += ones^T @ expT          4 heads col-packed, PSUM accum
  epilogue per q-block:
    recip = 1/sums ; broadcast to 32 partitions per head ; out = outT * recip
"""

import numpy as np

N, C, HH, WW = 4, 256, 64, 64
L = HH * WW            # 4096
NHEADS = 8
D = 32                 # head dim
HPC = 4                # heads per core
NCORES = 8
P = 128
QB = 512               # queries per block
NQB = L // QB          # 8
KCH = 128              # keys per chunk
NKC = L // KCH         # 32
SCALE = float(1.0 / np.sqrt(np.float32(D)))

_CACHE = {}


def _pe_T() -> np.ndarray:
    """Positional encoding transposed: [C, L] float32 (matches reference)."""
    pos = np.arange(L, dtype=np.float32)[None, :]            # (1, L)
    i = np.arange(C, dtype=np.float32)[:, None]              # (C, 1)
    angle = pos / np.power(
        np.float32(10000.0), (2.0 * np.floor(i / 2.0) / C).astype(np.float32)
    )
    pe = np.where(
        (np.arange(C, dtype=np.int64)[:, None] % 2) == 0, np.sin(angle), np.cos(angle)
    )
    return pe.astype(np.float32)


def build_nc():
    """Build the (shared-NEFF, SPMD) Bass module for one core."""
    import concourse.bacc as bacc
    import concourse.mybir as mybir
    import concourse.tile as tile

    f32 = mybir.dt.float32
    bf16 = mybir.dt.bfloat16
    EXP = mybir.ActivationFunctionType.Exp

    nc = bacc.Bacc("TRN2", target_bir_lowering=False, debug=False)

    xn = nc.dram_tensor("xn", [C, L], f32, kind="ExternalInput").ap()
    pet = nc.dram_tensor("pet", [C, L], f32, kind="ExternalInput").ap()
    w_qk = nc.dram_tensor("w_qk", [C, 2 * HPC * D], f32, kind="ExternalInput").ap()
    w_v = nc.dram_tensor("w_v", [C, HPC * D], f32, kind="ExternalInput").ap()
    out = nc.dram_tensor("out", [HPC * D, L], f32, kind="ExternalOutput").ap()

    with tile.TileContext(nc) as tc:
        with tc.tile_pool(name="persist", bufs=1) as persist:
            # ---- persistent SBUF tensors ----
            qT = persist.tile([P, L], bf16, tag="qT")    # [4h x 32d, L]
            kT = persist.tile([P, L], bf16, tag="kT")
            v_sb = persist.tile([P, L], bf16, tag="v")   # [keys%128, kc*128 + 32h+d]
            ones_t = persist.tile([P, D], bf16, tag="ones")
            nc.vector.memset(ones_t, 1.0)

            # ---- phase 1: xtT = x[n] + peT ; load weights ----
            with (
                tc.tile_pool(name="xt", bufs=1) as xt_pool,
                tc.tile_pool(name="ld", bufs=1) as ld_pool,
                tc.tile_pool(name="ppsum", bufs=2, space="PSUM") as ppsum,
            ):
                xtT = []
                for cc in range(2):
                    xnt = ld_pool.tile([P, L], f32, tag=f"xn{cc}")
                    pett = ld_pool.tile([P, L], f32, tag=f"pe{cc}")
                    nc.sync.dma_start(out=xnt, in_=xn[cc * P : (cc + 1) * P, :])
                    nc.sync.dma_start(out=pett, in_=pet[cc * P : (cc + 1) * P, :])
                    xt = xt_pool.tile([P, L], f32, tag=f"xtT{cc}")
                    nc.vector.tensor_add(xt, xnt, pett)
                    xtT.append(xt)

                wqk_sb, wv_sb = [], []
                for cc in range(2):
                    t = ld_pool.tile([P, 2 * HPC * D], f32, tag=f"wqk{cc}")
                    nc.sync.dma_start(out=t, in_=w_qk[cc * P : (cc + 1) * P, :])
                    wqk_sb.append(t)
                    t2 = ld_pool.tile([P, HPC * D], f32, tag=f"wv{cc}")
                    nc.sync.dma_start(out=t2, in_=w_v[cc * P : (cc + 1) * P, :])
                    wv_sb.append(t2)

                # ---- phase 2a: qT, kT = (W_qk chunk)^T @ xtT, cast bf16 ----
                for g, dest in enumerate((qT, kT)):
                    for lb in range(NQB):
                        ps = ppsum.tile([P, QB], f32, tag="proj", bufs=2)
                        for cc in range(2):
                            nc.tensor.matmul(
                                out=ps,
                                lhsT=wqk_sb[cc][:, g * P : (g + 1) * P],
                                rhs=xtT[cc][:, lb * QB : (lb + 1) * QB],
                                start=(cc == 0),
                                stop=(cc == 1),
                            )
                        nc.vector.tensor_copy(dest[:, lb * QB : (lb + 1) * QB], ps)

                # ---- phase 2b: v = xtT^T @ W_v, cast bf16 ----
                for kc in range(NKC):
                    ps = ppsum.tile([P, HPC * D], f32, tag="projv", bufs=2)
                    for cc in range(2):
                        nc.tensor.matmul(
                            out=ps,
                            lhsT=xtT[cc][:, kc * P : (kc + 1) * P],
                            rhs=wv_sb[cc],
                            start=(cc == 0),
                            stop=(cc == 1),
                        )
                    nc.vector.tensor_copy(v_sb[:, kc * P : (kc + 1) * P], ps)

            # ---- phase 3: attention ----
            with (
                tc.tile_pool(name="scpool", bufs=1, space="PSUM") as scpool,
                tc.tile_pool(name="accpool", bufs=1, space="PSUM") as accpool,
                tc.tile_pool(name="expool", bufs=1) as expool,
                tc.tile_pool(name="finpool", bufs=1) as finpool,
            ):
                for qb in range(NQB):
                    qsl = slice(qb * QB, (qb + 1) * QB)
                    att_ps = accpool.tile([P, QB], f32, tag="att", bufs=1)
                    sum_ps = accpool.tile([P, QB], f32, tag="sum", bufs=1)
                    for kc in range(NKC):
                        ksl = slice(kc * KCH, (kc + 1) * KCH)
                        sc = [
                            scpool.tile(
                                [P, 2 * QB], f32, tag="sc", bufs=3,
                                name=f"sc{qb}_{kc}_{t}",
                            )
                            for t in range(2)
                        ]
                        for h in range(HPC):
                            nc.tensor.matmul(
                                out=sc[h // 2][:, (h % 2) * QB : (h % 2 + 1) * QB],
                                lhsT=kT[32 * h : 32 * h + 32, ksl],
                                rhs=qT[32 * h : 32 * h + 32, qsl],
                                start=True,
                                stop=True,
                                tile_position=(32 * h, 0),
                            )
                        ex = [
                            expool.tile(
                                [P, 2 * QB], bf16, tag="ex", bufs=4,
                                name=f"ex{qb}_{kc}_{t}",
                            )
                            for t in range(2)
                        ]
                        for t in range(2):
                            nc.scalar.activation(ex[t], sc[t], EXP, scale=SCALE)
                        for h in range(HPC):
                            nc.tensor.matmul(
                                out=att_ps[32 * h : 32 * h + 32, :],
                                lhsT=v_sb[:, kc * P + 32 * h : kc * P + 32 * h + 32],
                                rhs=ex[h // 2][:, (h % 2) * QB : (h % 2 + 1) * QB],
                                start=(kc == 0),
                                stop=(kc == NKC - 1),
                                tile_position=(0, 32 * h),
                                skip_group_check=True,
                            )
                        for h in range(HPC):
                            # ones[128,32] stationary: each head's key-sum is
                            # broadcast across its 32 output partitions by the PE
                            nc.tensor.matmul(
                                out=sum_ps[32 * h : 32 * h + 32, :],
                                lhsT=ones_t,
                                rhs=ex[h // 2][:, (h % 2) * QB : (h % 2 + 1) * QB],
                                start=(kc == 0),
                                stop=(kc == NKC - 1),
                                tile_position=(0, 32 * h),
                                skip_group_check=True,
                            )
                    # epilogue: out = att * (1/sums)
                    recip = finpool.tile([P, QB], f32, tag="recip", bufs=2)
                    nc.vector.reciprocal_approx_fast(out=recip, in_=sum_ps)
                    o_sb = finpool.tile([P, QB], f32, tag="osb", bufs=2)
                    nc.vector.tensor_mul(o_sb, att_ps, recip)
                    nc.sync.dma_start(out=out[:, qsl], in_=o_sb)

    nc.compile()
    return nc


def _get_nc():
    if "nc" not in _CACHE:
        _CACHE["nc"] = build_nc()
    return _CACHE["nc"]


def make_in_maps(x: np.ndarray, W_qkv: np.ndarray):
    """Per-core input dicts."""
    x = np.ascontiguousarray(x, dtype=np.float32)
    W_qkv = np.ascontiguousarray(W_qkv, dtype=np.float32)
    pet = _pe_T()
    in_maps = []
    for c in range(NCORES):
        n = c // 2
        h0 = HPC * (c % 2)
        w_qk = np.concatenate(
            [
                W_qkv[:, h0 * D : h0 * D + HPC * D],
                W_qkv[:, C + h0 * D : C + h0 * D + HPC * D],
            ],
            axis=1,
        )
        w_v = W_qkv[:, 2 * C + h0 * D : 2 * C + h0 * D + HPC * D]
        in_maps.append(
            {
                "xn": np.ascontiguousarray(x[n].reshape(C, L)),
                "pet": pet,
                "w_qk": np.ascontiguousarray(w_qk),
                "w_v": np.ascontiguousarray(w_v),
            }
        )
    return in_maps


def assemble(results) -> np.ndarray:
    out = np.empty((N, C, L), dtype=np.float32)
    for c in range(NCORES):
        n = c // 2
        r0 = P * (c % 2)
        out[n, r0 : r0 + P, :] = results[c]["out"]
    return out.reshape(N, C, HH, WW)


def kernel(x: np.ndarray, W_qkv: np.ndarray) -> np.ndarray:
    from concourse.bass_utils import run_bass_kernel_spmd

    nc = _get_nc()
    in_maps = make_in_maps(x, W_qkv)
    res = run_bass_kernel_spmd(nc, in_maps, core_ids=list(range(NCORES)))
    return assemble(res.results)

